# revision 10
# baseline (speedup 1.0000x reference)
"""Trainium2 Bass kernel for GNN message-passing layer (8 NeuronCores).

Sharding: edges bucketed by dst-node range -> each core owns 12500 output
nodes and all edges pointing into them (zero collectives). Within a core,
edges are sorted by 128-node dst block; segment-sum is done as one-hot
matmuls on the PE accumulating into PSUM per block.

Math refactor: agg = segsum(gelu(x@W1+b1) @ W2 + b2) is linear past the
gelu, so W2/b2 fold into the update weights on the host:
  h = concat(nf, agg) @ W3 + b3
    = nf@W3a + segsum(h1)@(W2@W3b) + deg*(b2@W3b) + b3
Device computes h1 = gelu(x@W1+b1) per edge, segment-sums h1, and applies
folded weights in the per-block epilogue (matmul + LN + gelu + residual).
"""

import sys

for _p in ("/opt/pypackages", "/opt/trn_rl_repo", "/opt/trn_rl_repo/concourse"):
    if _p not in sys.path:
        sys.path.insert(0, _p)

from contextlib import ExitStack

import numpy as np
import ml_dtypes

import concourse.bass as bass
import concourse.bacc as bacc
import concourse.tile as tile
from concourse import mybir
from concourse.bass_utils import run_bass_kernel_spmd

N_NODES = 100000
HIDDEN = 64
EDGE_DIM = 16
N_CORES = 8
NPC = N_NODES // N_CORES          # 12500 nodes per core
BLK = 128                          # node block
NBLK = (NPC + BLK - 1) // BLK      # 98 blocks (12544 rows padded)
NPAD = NBLK * BLK                  # 12544
PAIR_E = 256                       # edges per pair (2 tiles of 128)
LN_EPS = 1e-6

f32 = mybir.dt.float32
bf16 = mybir.dt.bfloat16
i32 = mybir.dt.int32


def _host_shard(node_features, edge_features, edge_index):
    """Bucket edges by owning core / dst block; build padded per-core slabs."""
    src = np.asarray(edge_index[0], dtype=np.int64)
    dst = np.asarray(edge_index[1], dtype=np.int64)
    ef = np.asarray(edge_features, dtype=np.float32)

    core_of = dst // NPC
    per_core = []
    for c in range(N_CORES):
        m = np.nonzero(core_of == c)[0]
        d_rel = dst[m] - c * NPC
        blk = d_rel // BLK
        order = np.argsort(blk, kind="stable")
        m = m[order]
        d_rel = d_rel[order]
        blk = blk[order]
        counts = np.bincount(blk, minlength=NBLK)
        per_core.append((m, d_rel, counts))

    # pairs per block: same across cores (SPMD single program)
    pairs_b = np.zeros(NBLK, dtype=np.int64)
    for b in range(NBLK):
        mx = max(per_core[c][2][b] for c in range(N_CORES))
        pairs_b[b] = max(1, -(-int(mx) // PAIR_E))
    T = int(pairs_b.sum()) * 2          # 128-edge tiles total
    TE = T * 128

    tile_base = np.zeros(NBLK, dtype=np.int64)  # first tile index of block
    acc = 0
    for b in range(NBLK):
        tile_base[b] = acc
        acc += pairs_b[b] * 2

    shards = []
    for c in range(N_CORES):
        m, d_rel, counts = per_core[c]
        src_slab = np.zeros((128, T), dtype=np.int32)
        dst_slab = np.full((128, T), -1.0, dtype=np.float32)
        efT = np.zeros((16, TE), dtype=np.float32)
        off = 0
        for b in range(NBLK):
            n = int(counts[b])
            s0 = int(tile_base[b]) * 128      # first edge slot of block
            idx = m[off:off + n]
            off += n
            slots = s0 + np.arange(n)
            p, t = slots % 128, slots // 128
            src_slab[p, t] = src[idx].astype(np.int32)
            dst_slab[p, t] = (d_rel[off - n:off] - b * BLK).astype(np.float32)
            efT[:, slots] = ef[idx].T
        deg = np.bincount(np.asarray(d_rel, dtype=np.int64), minlength=NPAD
                          ).astype(np.float32)
        degext = np.empty((NBLK, 2, 128), dtype=np.float32)
        degext[:, 0, :] = deg[:NPAD].reshape(NBLK, 128)
        degext[:, 1, :] = 1.0
        shards.append(dict(src_slab=src_slab, dst_slab=dst_slab, efT=efT,
                           degext=degext))
    return shards, pairs_b, tile_base, T, TE


def _build_program(pairs_b, tile_base, T, TE):
    nc = bacc.Bacc("TRN2", target_bir_lowering=False, debug=False)
    nf = nc.declare_dram_parameter("nf", [N_NODES + 96, HIDDEN], f32, isOutput=False)
    nfT = nc.declare_dram_parameter("nfT", [HIDDEN, NPAD], f32, isOutput=False)
    nfres = nc.declare_dram_parameter("nfres", [NPAD, HIDDEN], f32, isOutput=False)
    efT_d = nc.declare_dram_parameter("efT", [EDGE_DIM, TE], f32, isOutput=False)
    src_d = nc.declare_dram_parameter("src_slab", [128, T], i32, isOutput=False)
    dst_d = nc.declare_dram_parameter("dst_slab", [128, T], f32, isOutput=False)
    deg_d = nc.declare_dram_parameter("degext", [NBLK, 2, 128], f32, isOutput=False)
    w1_d = nc.declare_dram_parameter("W1bf", [HIDDEN + EDGE_DIM, HIDDEN], bf16, isOutput=False)
    b1_d = nc.declare_dram_parameter("b1c", [HIDDEN, 1], f32, isOutput=False)
    w3a_d = nc.declare_dram_parameter("W3A", [128, HIDDEN], f32, isOutput=False)
    w3b_d = nc.declare_dram_parameter("W3B", [2, HIDDEN], f32, isOutput=False)
    lns_d = nc.declare_dram_parameter("lns_rep", [128, HIDDEN], f32, isOutput=False)
    lnb_d = nc.declare_dram_parameter("lnb_rep", [128, HIDDEN], f32, isOutput=False)
    iota_d = nc.declare_dram_parameter("iota_c", [128, 128], f32, isOutput=False)
    id_d = nc.declare_dram_parameter("id_c", [128, 128], f32, isOutput=False)
    out_d = nc.declare_dram_parameter("out", [NPAD, HIDDEN], f32, isOutput=True)

    GELU = mybir.ActivationFunctionType.Gelu_apprx_tanh

    with tile.TileContext(nc) as tc, ExitStack() as ctx:
        singles = ctx.enter_context(tc.tile_pool(name="singles", bufs=1))
        px = ctx.enter_context(tc.tile_pool(name="px", bufs=3))
        pxt = ctx.enter_context(tc.tile_pool(name="pxt", bufs=3))
        phs = ctx.enter_context(tc.tile_pool(name="phs", bufs=3))
        pems = ctx.enter_context(tc.tile_pool(name="pems", bufs=3))
        po = ctx.enter_context(tc.tile_pool(name="po", bufs=4))
        pps = ctx.enter_context(tc.tile_pool(name="pps", bufs=2, space="PSUM"))
        ph1 = ctx.enter_context(tc.tile_pool(name="ph1", bufs=2, space="PSUM"))
        pagg = ctx.enter_context(tc.tile_pool(name="pagg", bufs=2, space="PSUM"))
        phh = ctx.enter_context(tc.tile_pool(name="phh", bufs=1, space="PSUM"))
        pu = ctx.enter_context(tc.tile_pool(name="pu", bufs=2))
        pln = ctx.enter_context(tc.tile_pool(name="pln", bufs=4))
        pout = ctx.enter_context(tc.tile_pool(name="pout", bufs=3))

        # --- constants ---
        w1_sb = singles.tile([HIDDEN + EDGE_DIM, HIDDEN], bf16)
        nc.sync.dma_start(out=w1_sb, in_=w1_d[:])
        b1_sb = singles.tile([HIDDEN, 1], f32)
        nc.sync.dma_start(out=b1_sb, in_=b1_d[:])
        w3a_sb = singles.tile([128, HIDDEN], f32)
        nc.sync.dma_start(out=w3a_sb, in_=w3a_d[:])
        w3b_sb = singles.tile([2, HIDDEN], f32)
        nc.sync.dma_start(out=w3b_sb, in_=w3b_d[:])
        lns_sb = singles.tile([128, HIDDEN], f32)
        nc.sync.dma_start(out=lns_sb, in_=lns_d[:])
        lnb_sb = singles.tile([128, HIDDEN], f32)
        nc.sync.dma_start(out=lnb_sb, in_=lnb_d[:])
        src_sb = singles.tile([128, T], i32)
        nc.sync.dma_start(out=src_sb, in_=src_d[:])
        dst_sb = singles.tile([128, T], f32)
        nc.sync.dma_start(out=dst_sb, in_=dst_d[:])

        id32 = singles.tile([128, 128], f32)
        nc.sync.dma_start(out=id32, in_=id_d[:])
        idbf = singles.tile([128, 128], bf16)
        nc.vector.tensor_copy(out=idbf, in_=id32)
        iota_f = singles.tile([128, 128], f32)
        nc.sync.dma_start(out=iota_f, in_=iota_d[:])
        eps_sb = singles.tile([128, 1], f32)
        nc.vector.memset(eps_sb, LN_EPS)

        for b in range(NBLK):
            npair = int(pairs_b[b])
            tb = int(tile_base[b])
            aggT = pagg.tile([HIDDEN, 128], f32, tag="aggT")
            for q in range(npair):
                t0 = tb + 2 * q
                x32 = px.tile([128, 128], f32, tag="x32")
                for s2 in range(2):
                    nc.gpsimd.indirect_dma_start(
                        out=x32[:, 64 * s2:64 * s2 + 64],
                        out_offset=None,
                        in_=nf[:],
                        in_offset=bass.IndirectOffsetOnAxis(
                            ap=src_sb[:, t0 + s2:t0 + s2 + 1], axis=0),
                    )
                xt_ps = pps.tile([128, 128], f32, tag="xt_ps")
                nc.tensor.transpose(out=xt_ps, in_=x32, identity=id32)
                ef_sb = px.tile([EDGE_DIM, 256], f32, tag="ef_sb")
                nc.sync.dma_start(out=ef_sb,
                                  in_=efT_d[:, t0 * 128: t0 * 128 + 256])
                xt_sb = pxt.tile([HIDDEN + EDGE_DIM, 256], bf16, tag="xt_sb")
                nc.vector.tensor_copy(out=xt_sb[0:64, 0:128], in_=xt_ps[0:64, :])
                nc.vector.tensor_copy(out=xt_sb[0:64, 128:256], in_=xt_ps[64:128, :])
                nc.vector.tensor_copy(out=xt_sb[64:80, :], in_=ef_sb)
                h1_ps = ph1.tile([HIDDEN, 256], f32, tag="h1_ps")
                nc.tensor.matmul(h1_ps, lhsT=w1_sb, rhs=xt_sb,
                                 start=True, stop=True)
                h1_st = phs.tile([128, 128], bf16, tag="h1_st")
                nc.scalar.activation(out=h1_st[0:64, :], in_=h1_ps[:, 0:128],
                                     func=GELU, bias=b1_sb)
                nc.scalar.activation(out=h1_st[64:128, :], in_=h1_ps[:, 128:256],
                                     func=GELU, bias=b1_sb)
                em_ps = pps.tile([128, 128], bf16, tag="xt_ps")
                nc.tensor.transpose(out=em_ps, in_=h1_st, identity=idbf)
                em_sb = pems.tile([128, 128], bf16, tag="em_sb")
                nc.vector.tensor_copy(out=em_sb, in_=em_ps)
                for s in range(2):
                    t = t0 + s
                    o_t = po.tile([128, 128], bf16, tag="o_t")
                    nc.vector.tensor_tensor(
                        out=o_t, in0=dst_sb[:, t:t + 1].to_broadcast([128, 128]),
                        in1=iota_f, op=mybir.AluOpType.is_equal)
                    nc.tensor.matmul(
                        aggT, lhsT=em_sb[:, 64 * s:64 * s + 64], rhs=o_t,
                        start=(q == 0 and s == 0),
                        stop=(q == npair - 1 and s == 1))

            # ---- block epilogue ----
            nfT_sb = pu.tile([HIDDEN, 128], f32, tag="nfT_sb")
            nc.sync.dma_start(out=nfT_sb, in_=nfT[:, b * 128:(b + 1) * 128])
            updT = pu.tile([128, 128], f32, tag="updT")
            nc.vector.tensor_copy(out=updT[0:64, :], in_=nfT_sb)
            nc.vector.tensor_copy(out=updT[64:128, :], in_=aggT)
            extras = pu.tile([2, 128], f32, tag="extras")
            nc.sync.dma_start(out=extras, in_=deg_d[b])
            h_ps = phh.tile([128, HIDDEN], f32, tag="h_ps")
            nc.tensor.matmul(h_ps, lhsT=updT, rhs=w3a_sb, start=True, stop=False)
            nc.tensor.matmul(h_ps, lhsT=extras, rhs=w3b_sb, start=False, stop=True)
            bnst = pln.tile([128, 6], f32, tag="bnst")
            nc.vector.bn_stats(out=bnst, in_=h_ps)
            mv = pln.tile([128, 2], f32, tag="mv")
            nc.vector.bn_aggr(out=mv, in_=bnst)
            rstd = pln.tile([128, 1], f32, tag="rstd")
            nc.scalar.activation(out=rstd, in_=mv[:, 1:2],
                                 func=mybir.ActivationFunctionType.Sqrt,
                                 bias=eps_sb)
            nc.vector.reciprocal(out=rstd, in_=rstd)
            hn = pln.tile([128, HIDDEN], f32, tag="hn")
            nc.vector.tensor_tensor(out=hn, in0=h_ps,
                                    in1=mv[:, 0:1].to_broadcast([128, 64]),
                                    op=mybir.AluOpType.subtract)
            nc.vector.tensor_tensor(out=hn, in0=hn,
                                    in1=rstd[:, 0:1].to_broadcast([128, 64]),
                                    op=mybir.AluOpType.mult)
            nc.vector.tensor_tensor(out=hn, in0=hn, in1=lns_sb,
                                    op=mybir.AluOpType.mult)
            nc.vector.tensor_tensor(out=hn, in0=hn, in1=lnb_sb,
                                    op=mybir.AluOpType.add)
            nc.scalar.activation(out=hn, in_=hn, func=GELU)
            nf_blk = pout.tile([128, HIDDEN], f32, tag="nf_blk")
            nc.sync.dma_start(out=nf_blk, in_=nfres[b * 128:(b + 1) * 128, :])
            o_sb = pout.tile([128, HIDDEN], f32, tag="o_sb")
            nc.vector.tensor_tensor(out=o_sb, in0=hn, in1=nf_blk,
                                    op=mybir.AluOpType.add)
            nc.sync.dma_start(out=out_d[b * 128:(b + 1) * 128, :], in_=o_sb)
    nc.finalize()
    return nc


def kernel(node_features, edge_features, edge_index, W1, b1, W2, b2, W3, b3,
           ln_scale, ln_bias, _trace=False, _trace_kwargs=None):
    node_features = np.asarray(node_features, dtype=np.float32)
    edge_features = np.asarray(edge_features, dtype=np.float32)
    edge_index = np.asarray(edge_index)
    W1 = np.asarray(W1, dtype=np.float32)
    b1 = np.asarray(b1, dtype=np.float32)
    W2 = np.asarray(W2, dtype=np.float32)
    b2 = np.asarray(b2, dtype=np.float32)
    W3 = np.asarray(W3, dtype=np.float32)
    b3 = np.asarray(b3, dtype=np.float32)
    ln_scale = np.asarray(ln_scale, dtype=np.float32)
    ln_bias = np.asarray(ln_bias, dtype=np.float32)

    shards, pairs_b, tile_base, T, TE = _host_shard(
        node_features, edge_features, edge_index)
    nc = _build_program(pairs_b, tile_base, T, TE)

    nfpad = np.zeros((N_NODES + 96, HIDDEN), dtype=np.float32)
    nfpad[:N_NODES] = node_features
    W3a, W3b = W3[:HIDDEN], W3[HIDDEN:]
    W3A = np.concatenate([W3a, W2 @ W3b], axis=0).astype(np.float32)
    W3B = np.stack([b2 @ W3b, b3]).astype(np.float32)
    w1bf = W1.astype(ml_dtypes.bfloat16)
    b1c = b1.reshape(HIDDEN, 1).astype(np.float32)
    lns_rep = np.broadcast_to(ln_scale, (128, HIDDEN)).copy()
    lnb_rep = np.broadcast_to(ln_bias, (128, HIDDEN)).copy()
    iota_c = np.broadcast_to(np.arange(128, dtype=np.float32), (128, 128)).copy()
    id_c = np.eye(128, dtype=np.float32)

    in_maps = []
    for c in range(N_CORES):
        sh = shards[c]
        nfT_c = np.ascontiguousarray(nfpad[c * NPC: c * NPC + NPAD].T)
        nfres_c = np.ascontiguousarray(nfpad[c * NPC: c * NPC + NPAD])
        in_maps.append({
            "nf": nfpad, "nfT": nfT_c, "nfres": nfres_c,
            "efT": sh["efT"], "src_slab": sh["src_slab"],
            "dst_slab": sh["dst_slab"], "degext": sh["degext"],
            "W1bf": w1bf, "b1c": b1c, "W3A": W3A, "W3B": W3B,
            "lns_rep": lns_rep, "lnb_rep": lnb_rep,
            "iota_c": iota_c, "id_c": id_c,
        })

    res = run_bass_kernel_spmd(nc, in_maps, list(range(N_CORES)),
                               trace=_trace, **(_trace_kwargs or {}))
    out = np.concatenate([np.asarray(res.results[c]["out"])[:NPC]
                          for c in range(N_CORES)], axis=0)
    if _trace:
        return out, res
    return out
